# revision 47
# baseline (speedup 1.0000x reference)
"""Multi-head self-attention (b=4, n=2048, d=512, h=8, dh=64) on 8 trn2 cores.

Sharding: core c -> (batch b = c//2, head half hh = c%2). Each core computes
4 heads over the FULL sequence; the two cores of a batch produce partial
outputs (their heads' slice of the output projection) that the host SUMS,
adding the bias.

v3 schedule notes (vs v2 baseline):
  - ONE shared 3-buffer PSUM pool (6 banks) rotates S-score pairs AND all
    projection units, so the S stream runs 3 deep: S(g+3) only WARs on
    exp(g), letting the two exp engines (ACT true-exp / DVE Schraudolph)
    overlap fully instead of serializing the slot cadence.
  - AV accumulators for the head pair live in ONE [65, 2, 512] PSUM tile
    (2 banks).  Normalize is pair-batched: reciprocal_approx_fast reads the
    whole tile straight from PSUM (rows 0..63 produce garbage recips that
    are never read - only row 64, the ones-row denominator, is used), so
    the old per-head row-64 ACT extraction copies are gone.
  - The denominator broadcast runs on GpSimd; the two OT multiplies are
    emitted a few slots later so they never head-of-line block DVE exps.
  - Input DMA is reordered (Wq, xT[0:1024], Wk, ...) so the Q/K prelude
    projections start ~5us in instead of waiting for the full 2.8MB.
  - AV lags the S stream by 6 slots; the tail drains in ~10us.
"""

import sys

sys.path.insert(0, "/opt/trn_rl_repo")

from contextlib import ExitStack

import ml_dtypes
import numpy as np

import concourse.bass as bass
import concourse.tile as tile
from concourse import bacc, mybir
from concourse.bass import ts, ds
from concourse.bass_utils import run_bass_kernel_spmd

BF16 = mybir.dt.bfloat16
F32 = mybir.dt.float32
I16 = mybir.dt.int16

D = 512         # model dim
HL = 4          # heads per core
DH = 64
N = 2048        # full sequence per core
P = 128
KO = 4          # xT chunks of model dim
JT = 16         # kv tiles of 128
NB = 4          # q blocks of 512
SCALE = DH ** -0.5
# Schraudolph exp: bf16 bitcast of int16(x*128/ln2 + (127<<7) - C)
SCH_A = float(128.0 / np.log(2.0) * SCALE)
SCH_B = float(127 * 128 - 4)

# which j-slots of each 16-slot phase the DVE handles exp for (rest: ACT).
# Near-alternating: a run of consecutive same-engine exp slots pins the
# slot cadence to that engine's full exp duration (~1.1-1.2us) instead of
# the two-engine pipelined rate.  DVE takes r3 AND r4 so ACT has a 2-slot
# gap to run the finish chain's rs+accS PSUM evacuation without
# displacing any ACT exp.
DVE_EXP_SLOTS = {1, 3, 4, 6, 8, 10, 12, 14}
TT_DELAY = 6    # slots between finish_a1 (PSUM evacuation) and OT multiplies
AV_LAG = 5      # steady AV lag: slot r runs this phase's j = r - AV_LAG;
                # slots 0..4 run the previous phase's j = 11..15 (the AV
                # buffer release deadline is r5, after rs+accS complete).


def build_nc(finalize=True):
    nc = bacc.Bacc("TRN2", target_bir_lowering=False)

    xT_d = nc.dram_tensor("xT", [P, KO, N], BF16, kind="ExternalInput")
    Wq_d = nc.dram_tensor("Wq", [P, KO, 256], BF16, kind="ExternalInput")
    Wk_d = nc.dram_tensor("Wk", [P, KO, 256], BF16, kind="ExternalInput")
    Wv_d = nc.dram_tensor("Wv", [P, KO, 256], BF16, kind="ExternalInput")
    Wo_d = nc.dram_tensor("Wo", [P, 2, D], BF16, kind="ExternalInput")
    out_d = nc.dram_tensor("out", [N, D], F32, kind="ExternalOutput")

    with tile.TileContext(nc) as tc, ExitStack() as ctx:
        consts = ctx.enter_context(tc.tile_pool(name="consts", bufs=1))
        # ONE shared rotating PSUM pool: S pairs + q/k/v/o projection units.
        # 3 buffers x [128, 2, 512] f32 = 6 banks.
        big = ctx.enter_context(tc.tile_pool(name="big", bufs=3, space="PSUM"))
        # AV pair accumulator: [65, 2, 512] = 2 banks, single buffer.
        avp = ctx.enter_context(tc.tile_pool(name="avp", bufs=1, space="PSUM"))
        expp = ctx.enter_context(tc.tile_pool(name="expp", bufs=16))
        small = ctx.enter_context(tc.tile_pool(name="small", bufs=4))
        outp = ctx.enter_context(tc.tile_pool(name="outp", bufs=3))

        # ---- persistent SBUF tensors ----
        xT_sb = consts.tile([P, KO, N], BF16, tag="xT")
        Wq_sb = consts.tile([P, KO, 256], BF16, tag="Wq")
        Wk_sb = consts.tile([P, KO, 256], BF16, tag="Wk")
        Wv_sb = consts.tile([P, KO, 256], BF16, tag="Wv")
        Wo_sb = consts.tile([P, 2, D], BF16, tag="Wo")
        QT_sb = consts.tile([P, 2, N], BF16, tag="QT")
        KT_sb = consts.tile([P, 2, N], BF16, tag="KT")
        Vaug_sb = consts.tile([P, JT, HL, DH + 1], BF16, tag="Vaug")
        OT_sb = consts.tile([P, 2, N], BF16, tag="OT")

        # input DMAs ordered so the q/k prelude's data arrives first (the
        # SP issues one descriptor batch per ~0.8us and later transfers
        # compete for DMA engines, so order = priority)
        nc.sync.dma_start(Wq_sb[:], Wq_d[:])
        nc.sync.dma_start(xT_sb[:, :, 0:512], xT_d[:, :, 0:512])
        nc.sync.dma_start(Wk_sb[:], Wk_d[:])
        nc.sync.dma_start(xT_sb[:, :, 512:1024], xT_d[:, :, 512:1024])
        nc.sync.dma_start(Wv_sb[:], Wv_d[:])
        nc.sync.dma_start(xT_sb[:, :, 1024:1536], xT_d[:, :, 1024:1536])
        nc.sync.dma_start(xT_sb[:, :, 1536:2048], xT_d[:, :, 1536:2048])
        nc.sync.dma_start(Wo_sb[:], Wo_d[:])

        nc.vector.memset(Vaug_sb[:, :, :, DH : DH + 1], 1.0)

        # spin the PE so HAM unthrottles before the first real matmuls
        junk = small.tile([64, 64], BF16, tag="junk")
        nc.vector.memset(junk[:], 0.0)
        # spins must cover the whole input-DMA wait (~14us): if the PE goes
        # idle >3.4us before the prelude projections, HAM re-throttles and
        # the q/k/first-S matmuls all run at half clock
        wp = big.tile([P, 2, 512], F32, tag="big", name="warm")
        for _ in range(110):
            nc.tensor.matmul(wp[0:64, 0, 0:64], lhsT=junk[:], rhs=junk[:],
                             start=True, stop=True)
        # touch the exp table early so ACT_TABLE_LOAD overlaps the DMAs
        warm = small.tile([1, 8], F32, tag="warm")
        nc.scalar.activation(warm[:], junk[0:1, 0:8],
                             mybir.ActivationFunctionType.Exp)

        def proj_part(W_sb, T_sb, o, cb, dve_cast=False):
            """Single 512-col projection part (prelude granularity): only
            needs xT columns [cb*512, cb*512+512), so it can start as soon
            as that input DMA block lands."""
            pp = big.tile([P, 2, 512], F32, tag="big",
                          name=f"pt{id(W_sb)%97}_{o}_{cb}")
            for k in range(KO):
                nc.tensor.matmul(
                    pp[:, 0, :],
                    lhsT=W_sb[:, k, ts(o, P)],
                    rhs=xT_sb[:, k, ts(cb, 512)],
                    start=(k == 0),
                    stop=(k == KO - 1),
                )
            if dve_cast:
                nc.vector.tensor_copy(T_sb[:, o, ts(cb, 512)], pp[:, 0, :])
            else:
                nc.scalar.activation(T_sb[:, o, ts(cb, 512)], pp[:, 0, :],
                                     mybir.ActivationFunctionType.Copy)

        def v_unit(jj, dve_cast=False):
            # two kv j-tiles (256 output cols each) in one PSUM buffer
            vp = big.tile([P, 2, 512], F32, tag="big", name=f"vp{jj}")
            for m in range(2):
                for k in range(KO):
                    nc.tensor.matmul(
                        vp[:, m, 0:256],
                        lhsT=xT_sb[:, k, ds((2 * jj + m) * P, P)],
                        rhs=Wv_sb[:, k, :],
                        start=(k == 0),
                        stop=(k == KO - 1),
                    )
            dst = Vaug_sb[:, 2 * jj : 2 * jj + 2, :, 0:DH]
            src = vp[:, :, 0:256].rearrange("p m (h d) -> p m h d", h=HL)
            if dve_cast:
                nc.vector.tensor_copy(dst, src)
            else:
                nc.scalar.activation(dst, src,
                                     mybir.ActivationFunctionType.Copy)

        # ---- attention stream state ----
        av_tiles = {}     # (i, n) -> psum pair accumulator [65, 2, 512]
        exp_tiles = {}    # (i, n, j) -> E tile [128, 2, 512] bf16

        def s_pair(i, n, j):
            """Score pair matmuls (heads 2i, 2i+1) for q block n, kv tile j."""
            sp = big.tile([P, 2, 512], F32, tag="big", name=f"sp{i}_{n}_{j}")
            nc.tensor.matmul(
                sp[:, 0, :],
                lhsT=KT_sb[0:64, i, ts(j, P)],
                rhs=QT_sb[0:64, i, ts(n, 512)],
                start=True, stop=True,
                tile_position=(0, 0),
            )
            nc.tensor.matmul(
                sp[:, 1, :],
                lhsT=KT_sb[64:128, i, ts(j, P)],
                rhs=QT_sb[64:128, i, ts(n, 512)],
                start=True, stop=True,
                tile_position=(64, 0),
            )
            return sp

        def s_exp(i, n, j, sp):
            eb = expp.tile([P, 2, 512], BF16, tag="expS", name=f"eb{i}_{n}_{j}")
            if j in DVE_EXP_SLOTS:
                nc.vector.tensor_scalar(
                    eb[:].bitcast(I16), sp[:], SCH_A, SCH_B,
                    mybir.AluOpType.mult, mybir.AluOpType.add,
                )
            else:
                nc.scalar.activation(
                    eb[:], sp[:], mybir.ActivationFunctionType.Exp,
                    scale=SCALE,
                )
            exp_tiles[(i, n, j)] = eb

        def av(i, n, j, s):
            """Accumulate [V|1]^T E for head 2i+s into the pair PSUM tile."""
            eb = exp_tiles[(i, n, j)]
            h = 2 * i + s
            if j == 0 and s == 0:
                av_tiles[(i, n)] = avp.tile(
                    [DH + 1, 2, 512], F32, tag="avp", name=f"av{i}_{n}"
                )
            nc.tensor.matmul(
                av_tiles[(i, n)][:, s, :],
                lhsT=Vaug_sb[:, j, h, :],
                rhs=eb[:, s, :],
                start=(j == 0),
                stop=(j == JT - 1),
                skip_group_check=True,
            )
            if s == 1:
                del exp_tiles[(i, n, j)]

        def av_finish_a1(i, n):
            """Evacuate the AV pair PSUM tile (releases the single avp
            buffer): row 64 of each bank = ones-column output (softmax
            denominator) to rs (PSUM reads may start at partition 64;
            partition_broadcast later needs the row on partition 0), AV
            values to SBUF bf16.  Both on ACT, in the phase window where
            DVE carries the exps."""
            acc = av_tiles.pop((i, n))
            rs = small.tile([1, 2, 512], F32, tag="rs", name=f"rs{i}_{n}")
            nc.scalar.activation(rs[:], acc[DH : DH + 1, :, :],
                                 mybir.ActivationFunctionType.Copy)
            accS = small.tile([DH, 2, 512], BF16, tag="accS",
                              name=f"accS{i}_{n}")
            nc.scalar.activation(accS[:], acc[0:DH, :, :],
                                 mybir.ActivationFunctionType.Copy)
            return rs, accS

        def av_finish_a2(i, n, rs):
            """Reciprocal + broadcast, emitted 2 slots later so the recip
            never sits mid-way through DVE's exp run."""
            rc = small.tile([1, 2, 512], F32, tag="rc", name=f"rc{i}_{n}")
            nc.vector.reciprocal_approx_fast(rc[:], rs[:])
            rb = small.tile([DH, 2, 512], F32, tag="rb", name=f"rb{i}_{n}")
            nc.gpsimd.partition_broadcast(rb[:], rc[:])
            return rb

        def av_finish_b(i, n, accS, rb):
            """OT multiplies for the pair (from the SBUF copy)."""
            for s in range(2):
                nc.vector.tensor_mul(
                    OT_sb[64 * s : 64 * s + DH, i, ts(n, 512)],
                    accS[:, s, :],
                    rb[:, s, :],
                )

        out_r = out_d.rearrange("(t p) d -> p t d", p=P)

        def o_unit(t, dve_copy=False):
            """Output projection for q tile t (128 rows) + DMA out."""
            op = big.tile([P, 2, 512], F32, tag="big", name=f"op{t}")
            for c in range(2):
                nc.tensor.matmul(
                    op[:, 0, :],
                    lhsT=OT_sb[:, c, ts(t, P)],
                    rhs=Wo_sb[:, c, :],
                    start=(c == 0),
                    stop=(c == 1),
                )
            ot = outp.tile([P, D], F32, tag="out")
            if dve_copy:
                nc.vector.tensor_copy(ot[:], op[:, 0, :])
            else:
                nc.scalar.activation(ot[:], op[:, 0, :],
                                     mybir.ActivationFunctionType.Copy)
            nc.sync.dma_start(out_r[:, t, :], ot[:])

        # ---- schedule ----
        extras = {}

        def addx(g, fn):
            extras.setdefault(g, []).append(fn)

        phases = [(i, n) for i in range(2) for n in range(NB)]
        total = len(phases) * JT

        # phase (0,0): V units at odd slots; K/Q 512-col parts at even
        # slots, ordered by when the S/AV streams need them (K cols c*512
        # feed S slots j = 4c..4c+3).
        for jj in range(8):
            addx(2 * jj + 1,
                 lambda jj=jj, e=(jj % 2 == 1): v_unit(jj, dve_cast=e))
        addx(0, lambda: proj_part(Wk_sb, KT_sb, 0, 1))
        addx(2, lambda: proj_part(Wk_sb, KT_sb, 0, 2))
        addx(4, lambda: proj_part(Wk_sb, KT_sb, 0, 3))
        addx(6, lambda: proj_part(Wq_sb, QT_sb, 0, 1))
        # phases (0,1)/(0,2): remaining Q/K as 512-col parts (short PSUM
        # pool holds - a full 8-matmul unit stalls the 3-deep S rotation)
        parts = [(Wq_sb, QT_sb, 0, 2), (Wq_sb, QT_sb, 0, 3),
                 (Wq_sb, QT_sb, 1, 0), (Wq_sb, QT_sb, 1, 1),
                 (Wk_sb, KT_sb, 1, 0), (Wk_sb, KT_sb, 1, 1),
                 (Wq_sb, QT_sb, 1, 2), (Wq_sb, QT_sb, 1, 3),
                 (Wk_sb, KT_sb, 1, 2), (Wk_sb, KT_sb, 1, 3)]
        for idx, (w, t_, o, cb) in enumerate(parts):
            addx(16 + 1 + 4 * idx,
                 lambda w=w, t_=t_, o=o, cb=cb, e=(idx % 2 == 0):
                     proj_part(w, t_, o, cb, dve_cast=e))

        # O-projection of q block m during pair-1 phase m+1 (needs
        # finish(0,m) [pair-0 phases] and finish(1,m) [previous phase];
        # after r11 so they never wait on the r8 OT multiplies in-queue).
        for m in range(NB - 1):
            pbase = (4 + m + 1) * JT
            for t in range(4):
                addx(pbase + 12 + t,
                     lambda t=t, m=m, e=(t == 3):
                         o_unit(4 * m + t, dve_copy=e))

        # prelude: minimum to start the stream (first q/k 512-col parts
        # only need xT cols 0:512 = the first input DMA block)
        proj_part(Wq_sb, QT_sb, 0, 0, dve_cast=True)
        proj_part(Wk_sb, KT_sb, 0, 0, dve_cast=False)

        # flat stream: one S pair + one AV pair per slot, continuous (no
        # PE idle gaps -> HAM stays warm).  Slot r of phase p runs AV for
        # this phase's j = r-4, or the PREVIOUS phase's j = 12..15 for
        # r = 0..3.  The finish chain is emitted at slot r=3 BEFORE that
        # slot's exp so its PSUM reads jump ahead in the ACT/DVE queues
        # and the single AV buffer frees just in time for j=0 at r=4.
        def slot_avs(g):
            """Old phase's j=11..15 in slots r0..r3 (doubled at r0 so the
            finish chain can start at r3); r4 is the ACT gap where rs+accS
            run; this phase's j=0..10 at r5..r15."""
            p, r = g // JT, g % JT
            if r < 4:
                if p == 0:
                    return []
                prev = phases[p - 1]
                return [(prev, j) for j in ([11, 12] if r == 0 else [12 + r])]
            if r == 4:
                return []
            return [(phases[p], r - AV_LAG)]

        def finish(ia, na, g):
            rs, accS = av_finish_a1(ia, na)
            st = {}
            def a2():
                st["rb"] = av_finish_a2(ia, na, rs)
            addx(g + 2, a2)
            addx(g + TT_DELAY,
                 lambda: av_finish_b(ia, na, accS, st["rb"]))

        for g in range(total):
            i, n = phases[g // JT]
            sp = s_pair(i, n, g % JT)
            avs = slot_avs(g)
            fin = any(j == JT - 1 for _, j in avs)
            if fin:
                for (pi, j) in avs:
                    av(pi[0], pi[1], j, 0)
                    av(pi[0], pi[1], j, 1)
                (ia, na), _ = avs[-1]
                finish(ia, na, g)
            s_exp(i, n, g % JT, sp)
            if not fin:
                for (pi, j) in avs:
                    av(pi[0], pi[1], j, 0)
                    av(pi[0], pi[1], j, 1)
            for fn in extras.get(g, []):
                fn()

        # ---- tail ----
        # Pre-start c=0 (pair-0 OT, ready since phase 3) of three block-3
        # output projections on the freeing S pool; their accumulation
        # groups stay open across the AV drain (different PSUM banks).
        pre = []
        for t in (12, 13, 14):
            op = big.tile([P, 2, 512], F32, tag="big", name=f"op{t}")
            nc.tensor.matmul(op[:, 0, :], lhsT=OT_sb[:, 0, ts(t, P)],
                             rhs=Wo_sb[:, 0, :], start=True, stop=False,
                             skip_group_check=True)
            pre.append((t, op))
        # drain last phase's AVs
        ia, na = phases[-1]
        for j in (11, 12, 13, 14, 15):
            av(ia, na, j, 0)
            av(ia, na, j, 1)
        # fast finish: no SBUF evacuation (nothing else needs the AV
        # buffer), recip immediately, OT multiplies straight from PSUM
        acc = av_tiles.pop((ia, na))
        rs = small.tile([1, 2, 512], F32, tag="rs", name="rs_tail")
        nc.scalar.activation(rs[:], acc[DH : DH + 1, :, :],
                             mybir.ActivationFunctionType.Copy)
        rc = small.tile([1, 2, 512], F32, tag="rc", name="rc_tail")
        nc.vector.reciprocal_approx_fast(rc[:], rs[:])
        rb = small.tile([DH, 2, 512], F32, tag="rb", name="rb_tail")
        nc.gpsimd.partition_broadcast(rb[:], rc[:])
        for s in range(2):
            nc.vector.tensor_mul(
                OT_sb[64 * s : 64 * s + DH, ia, ts(na, 512)],
                acc[0:DH, s, :],
                rb[:, s, :],
            )
        for g in range(total, total + TT_DELAY + 2):
            for fn in extras.get(g, []):
                fn()
        # complete the pre-started units (c=1 = fresh pair-1 OT) + o15
        for k, (t, op) in enumerate(pre):
            nc.tensor.matmul(op[:, 0, :], lhsT=OT_sb[:, 1, ts(t, P)],
                             rhs=Wo_sb[:, 1, :], start=False, stop=True,
                             skip_group_check=True)
            ot = outp.tile([P, D], F32, tag="out")
            if k % 2 == 1:
                nc.vector.tensor_copy(ot[:], op[:, 0, :])
            else:
                nc.scalar.activation(ot[:], op[:, 0, :],
                                     mybir.ActivationFunctionType.Copy)
            nc.sync.dma_start(out_r[:, t, :], ot[:])
        o_unit(15, dve_copy=True)

    if finalize:
        nc.finalize()
    return nc


_NC_CACHE = None


def _get_nc():
    global _NC_CACHE
    if _NC_CACHE is None:
        _NC_CACHE = build_nc()
    return _NC_CACHE


def _chunked(w):
    """[512, M] -> [128, 4, M] with row r at [r % 128, r // 128]."""
    m = w.shape[1]
    return np.ascontiguousarray(
        w.reshape(w.shape[0] // P, P, m).transpose(1, 0, 2)
    )


def make_in_maps(x, Wq, Wkv, Wo, bo):
    bf = ml_dtypes.bfloat16
    Wq = np.asarray(Wq, np.float32)
    Wkv = np.asarray(Wkv, np.float32)
    Wo = np.asarray(Wo, np.float32)
    xTs = []
    for b in range(4):
        xTs.append(_chunked(np.asarray(x[b], np.float32).T).astype(bf))
    in_maps = []
    for c in range(8):
        b, hh = divmod(c, 2)
        cols = slice(hh * 256, (hh + 1) * 256)
        in_maps.append({
            "xT": xTs[b],
            "Wq": _chunked(Wq[:, cols]).astype(bf),
            "Wk": _chunked(Wkv[:, :D][:, cols]).astype(bf),
            "Wv": _chunked(Wkv[:, D:][:, cols]).astype(bf),
            "Wo": _chunked(Wo[hh * 256 : (hh + 1) * 256, :]).astype(bf),
        })
    return in_maps


def gather_out(results, x, bo):
    b_total = x.shape[0]
    bo = np.asarray(bo, np.float32)
    out = np.empty((b_total, N, D), np.float32)
    for b in range(b_total):
        out[b] = results[2 * b]["out"] + results[2 * b + 1]["out"] + bo
    return out


def kernel(x, Wq, Wkv, Wo, bo, trace=False):
    nc = _get_nc()
    in_maps = make_in_maps(x, Wq, Wkv, Wo, bo)
    res = run_bass_kernel_spmd(nc, in_maps, core_ids=list(range(8)), trace=trace)
    out = gather_out(res.results, np.asarray(x), bo)
    if trace:
        kernel.last_exec_time_ns = res.exec_time_ns
    return out


kernel.last_exec_time_ns = None


# revision 48
# speedup vs baseline: 1.0264x; 1.0264x over previous
"""Multi-head self-attention (b=4, n=2048, d=512, h=8, dh=64) on 8 trn2 cores.

Sharding: core c -> (batch b = c//2, head half hh = c%2). Each core computes
4 heads over the FULL sequence; the two cores of a batch produce partial
outputs (their heads' slice of the output projection) that the host SUMS,
adding the bias.

v3 schedule notes (vs v2 baseline):
  - ONE shared 3-buffer PSUM pool (6 banks) rotates S-score pairs AND all
    projection units, so the S stream runs 3 deep: S(g+3) only WARs on
    exp(g), letting the two exp engines (ACT true-exp / DVE Schraudolph)
    overlap fully instead of serializing the slot cadence.
  - AV accumulators for the head pair live in ONE [65, 2, 512] PSUM tile
    (2 banks).  Normalize is pair-batched: reciprocal_approx_fast reads the
    whole tile straight from PSUM (rows 0..63 produce garbage recips that
    are never read - only row 64, the ones-row denominator, is used), so
    the old per-head row-64 ACT extraction copies are gone.
  - The denominator broadcast runs on GpSimd; the two OT multiplies are
    emitted a few slots later so they never head-of-line block DVE exps.
  - Input DMA is reordered (Wq, xT[0:1024], Wk, ...) so the Q/K prelude
    projections start ~5us in instead of waiting for the full 2.8MB.
  - AV lags the S stream by 6 slots; the tail drains in ~10us.
"""

import sys

sys.path.insert(0, "/opt/trn_rl_repo")

from contextlib import ExitStack

import ml_dtypes
import numpy as np

import concourse.bass as bass
import concourse.tile as tile
from concourse import bacc, mybir
from concourse.bass import ts, ds
from concourse.bass_utils import run_bass_kernel_spmd

BF16 = mybir.dt.bfloat16
F32 = mybir.dt.float32
I16 = mybir.dt.int16

D = 512         # model dim
HL = 4          # heads per core
DH = 64
N = 2048        # full sequence per core
P = 128
KO = 4          # xT chunks of model dim
JT = 16         # kv tiles of 128
NB = 4          # q blocks of 512
SCALE = DH ** -0.5
# Schraudolph exp: bf16 bitcast of int16(x*128/ln2 + (127<<7) - C)
SCH_A = float(128.0 / np.log(2.0) * SCALE)
SCH_B = float(127 * 128 - 4)

# which j-slots of each 16-slot phase the DVE handles exp for (rest: ACT).
# DVE covers the finish-chain window (r2..r5) so the rs/accS PSUM
# evacuation on ACT does not displace ACT exps and stall the S rotation.
DVE_EXP_SLOTS = {1, 2, 3, 4, 5, 13, 15}
TT_DELAY = 6    # slots between finish_a1 (PSUM evacuation) and OT multiplies
AV_LAG = 4      # steady AV lag: slot r runs this phase's j = r - AV_LAG;
                # slots 0..3 run the previous phase's j = 12..15.


def build_nc(finalize=True):
    nc = bacc.Bacc("TRN2", target_bir_lowering=False)

    xT_d = nc.dram_tensor("xT", [P, KO, N], BF16, kind="ExternalInput")
    Wq_d = nc.dram_tensor("Wq", [P, KO, 256], BF16, kind="ExternalInput")
    Wk_d = nc.dram_tensor("Wk", [P, KO, 256], BF16, kind="ExternalInput")
    Wv_d = nc.dram_tensor("Wv", [P, KO, 256], BF16, kind="ExternalInput")
    Wo_d = nc.dram_tensor("Wo", [P, 2, D], BF16, kind="ExternalInput")
    out_d = nc.dram_tensor("out", [N, D], F32, kind="ExternalOutput")

    with tile.TileContext(nc) as tc, ExitStack() as ctx:
        consts = ctx.enter_context(tc.tile_pool(name="consts", bufs=1))
        # ONE shared rotating PSUM pool: S pairs + q/k/v/o projection units.
        # 3 buffers x [128, 2, 512] f32 = 6 banks.
        big = ctx.enter_context(tc.tile_pool(name="big", bufs=3, space="PSUM"))
        # AV pair accumulator: [65, 2, 512] = 2 banks, single buffer.
        avp = ctx.enter_context(tc.tile_pool(name="avp", bufs=1, space="PSUM"))
        expp = ctx.enter_context(tc.tile_pool(name="expp", bufs=16))
        small = ctx.enter_context(tc.tile_pool(name="small", bufs=4))
        outp = ctx.enter_context(tc.tile_pool(name="outp", bufs=3))

        # ---- persistent SBUF tensors ----
        xT_sb = consts.tile([P, KO, N], BF16, tag="xT")
        Wq_sb = consts.tile([P, KO, 256], BF16, tag="Wq")
        Wk_sb = consts.tile([P, KO, 256], BF16, tag="Wk")
        Wv_sb = consts.tile([P, KO, 256], BF16, tag="Wv")
        Wo_sb = consts.tile([P, 2, D], BF16, tag="Wo")
        QT_sb = consts.tile([P, 2, N], BF16, tag="QT")
        KT_sb = consts.tile([P, 2, N], BF16, tag="KT")
        Vaug_sb = consts.tile([P, JT, HL, DH + 1], BF16, tag="Vaug")
        OT_sb = consts.tile([P, 2, N], BF16, tag="OT")

        # input DMAs ordered so the q/k prelude's data arrives first (the
        # SP issues one descriptor batch per ~0.8us and later transfers
        # compete for DMA engines, so order = priority)
        nc.sync.dma_start(Wq_sb[:], Wq_d[:])
        nc.sync.dma_start(xT_sb[:, :, 0:512], xT_d[:, :, 0:512])
        nc.sync.dma_start(Wk_sb[:], Wk_d[:])
        nc.sync.dma_start(xT_sb[:, :, 512:1024], xT_d[:, :, 512:1024])
        nc.sync.dma_start(Wv_sb[:], Wv_d[:])
        nc.sync.dma_start(xT_sb[:, :, 1024:1536], xT_d[:, :, 1024:1536])
        nc.sync.dma_start(xT_sb[:, :, 1536:2048], xT_d[:, :, 1536:2048])
        nc.sync.dma_start(Wo_sb[:], Wo_d[:])

        nc.vector.memset(Vaug_sb[:, :, :, DH : DH + 1], 1.0)

        # spin the PE so HAM unthrottles before the first real matmuls
        junk = small.tile([64, 64], BF16, tag="junk")
        nc.vector.memset(junk[:], 0.0)
        # spins must cover the whole input-DMA wait (~14us): if the PE goes
        # idle >3.4us before the prelude projections, HAM re-throttles and
        # the q/k/first-S matmuls all run at half clock
        wp = big.tile([P, 2, 512], F32, tag="big", name="warm")
        for _ in range(110):
            nc.tensor.matmul(wp[0:64, 0, 0:64], lhsT=junk[:], rhs=junk[:],
                             start=True, stop=True)
        # touch the exp table early so ACT_TABLE_LOAD overlaps the DMAs
        warm = small.tile([1, 8], F32, tag="warm")
        nc.scalar.activation(warm[:], junk[0:1, 0:8],
                             mybir.ActivationFunctionType.Exp)

        def proj_part(W_sb, T_sb, o, cb, dve_cast=False):
            """Single 512-col projection part (prelude granularity): only
            needs xT columns [cb*512, cb*512+512), so it can start as soon
            as that input DMA block lands."""
            pp = big.tile([P, 2, 512], F32, tag="big",
                          name=f"pt{id(W_sb)%97}_{o}_{cb}")
            for k in range(KO):
                nc.tensor.matmul(
                    pp[:, 0, :],
                    lhsT=W_sb[:, k, ts(o, P)],
                    rhs=xT_sb[:, k, ts(cb, 512)],
                    start=(k == 0),
                    stop=(k == KO - 1),
                )
            if dve_cast:
                nc.vector.tensor_copy(T_sb[:, o, ts(cb, 512)], pp[:, 0, :])
            else:
                nc.scalar.activation(T_sb[:, o, ts(cb, 512)], pp[:, 0, :],
                                     mybir.ActivationFunctionType.Copy)

        def v_unit(jj, dve_cast=False):
            # two kv j-tiles (256 output cols each) in one PSUM buffer
            vp = big.tile([P, 2, 512], F32, tag="big", name=f"vp{jj}")
            for m in range(2):
                for k in range(KO):
                    nc.tensor.matmul(
                        vp[:, m, 0:256],
                        lhsT=xT_sb[:, k, ds((2 * jj + m) * P, P)],
                        rhs=Wv_sb[:, k, :],
                        start=(k == 0),
                        stop=(k == KO - 1),
                    )
            dst = Vaug_sb[:, 2 * jj : 2 * jj + 2, :, 0:DH]
            src = vp[:, :, 0:256].rearrange("p m (h d) -> p m h d", h=HL)
            if dve_cast:
                nc.vector.tensor_copy(dst, src)
            else:
                nc.scalar.activation(dst, src,
                                     mybir.ActivationFunctionType.Copy)

        # ---- attention stream state ----
        av_tiles = {}     # (i, n) -> psum pair accumulator [65, 2, 512]
        exp_tiles = {}    # (i, n, j) -> E tile [128, 2, 512] bf16

        def s_pair(i, n, j):
            """Score pair matmuls (heads 2i, 2i+1) for q block n, kv tile j."""
            sp = big.tile([P, 2, 512], F32, tag="big", name=f"sp{i}_{n}_{j}")
            nc.tensor.matmul(
                sp[:, 0, :],
                lhsT=KT_sb[0:64, i, ts(j, P)],
                rhs=QT_sb[0:64, i, ts(n, 512)],
                start=True, stop=True,
                tile_position=(0, 0),
            )
            nc.tensor.matmul(
                sp[:, 1, :],
                lhsT=KT_sb[64:128, i, ts(j, P)],
                rhs=QT_sb[64:128, i, ts(n, 512)],
                start=True, stop=True,
                tile_position=(64, 0),
            )
            return sp

        def s_exp(i, n, j, sp):
            eb = expp.tile([P, 2, 512], BF16, tag="expS", name=f"eb{i}_{n}_{j}")
            if j in DVE_EXP_SLOTS:
                nc.vector.tensor_scalar(
                    eb[:].bitcast(I16), sp[:], SCH_A, SCH_B,
                    mybir.AluOpType.mult, mybir.AluOpType.add,
                )
            else:
                nc.scalar.activation(
                    eb[:], sp[:], mybir.ActivationFunctionType.Exp,
                    scale=SCALE,
                )
            exp_tiles[(i, n, j)] = eb

        def av(i, n, j, s):
            """Accumulate [V|1]^T E for head 2i+s into the pair PSUM tile."""
            eb = exp_tiles[(i, n, j)]
            h = 2 * i + s
            if j == 0 and s == 0:
                av_tiles[(i, n)] = avp.tile(
                    [DH + 1, 2, 512], F32, tag="avp", name=f"av{i}_{n}"
                )
            nc.tensor.matmul(
                av_tiles[(i, n)][:, s, :],
                lhsT=Vaug_sb[:, j, h, :],
                rhs=eb[:, s, :],
                start=(j == 0),
                stop=(j == JT - 1),
                skip_group_check=True,
            )
            if s == 1:
                del exp_tiles[(i, n, j)]

        def av_finish_a1(i, n):
            """Evacuate the AV pair PSUM tile (releases the single avp
            buffer): row 64 of each bank = ones-column output (softmax
            denominator) to rs (PSUM reads may start at partition 64;
            partition_broadcast later needs the row on partition 0), AV
            values to SBUF bf16.  Both on ACT, in the phase window where
            DVE carries the exps."""
            acc = av_tiles.pop((i, n))
            rs = small.tile([1, 2, 512], F32, tag="rs", name=f"rs{i}_{n}")
            nc.scalar.activation(rs[:], acc[DH : DH + 1, :, :],
                                 mybir.ActivationFunctionType.Copy)
            accS = small.tile([DH, 2, 512], BF16, tag="accS",
                              name=f"accS{i}_{n}")
            nc.scalar.activation(accS[:], acc[0:DH, :, :],
                                 mybir.ActivationFunctionType.Copy)
            return rs, accS

        def av_finish_a2(i, n, rs):
            """Reciprocal + broadcast, emitted 2 slots later so the recip
            never sits mid-way through DVE's exp run."""
            rc = small.tile([1, 2, 512], F32, tag="rc", name=f"rc{i}_{n}")
            nc.vector.reciprocal_approx_fast(rc[:], rs[:])
            rb = small.tile([DH, 2, 512], F32, tag="rb", name=f"rb{i}_{n}")
            nc.gpsimd.partition_broadcast(rb[:], rc[:])
            return rb

        def av_finish_b(i, n, accS, rb):
            """OT multiplies for the pair (from the SBUF copy)."""
            for s in range(2):
                nc.vector.tensor_mul(
                    OT_sb[64 * s : 64 * s + DH, i, ts(n, 512)],
                    accS[:, s, :],
                    rb[:, s, :],
                )

        out_r = out_d.rearrange("(t p) d -> p t d", p=P)

        def o_unit(t, dve_copy=False):
            """Output projection for q tile t (128 rows) + DMA out."""
            op = big.tile([P, 2, 512], F32, tag="big", name=f"op{t}")
            for c in range(2):
                nc.tensor.matmul(
                    op[:, 0, :],
                    lhsT=OT_sb[:, c, ts(t, P)],
                    rhs=Wo_sb[:, c, :],
                    start=(c == 0),
                    stop=(c == 1),
                )
            ot = outp.tile([P, D], F32, tag="out")
            if dve_copy:
                nc.vector.tensor_copy(ot[:], op[:, 0, :])
            else:
                nc.scalar.activation(ot[:], op[:, 0, :],
                                     mybir.ActivationFunctionType.Copy)
            nc.sync.dma_start(out_r[:, t, :], ot[:])

        # ---- schedule ----
        extras = {}

        def addx(g, fn):
            extras.setdefault(g, []).append(fn)

        phases = [(i, n) for i in range(2) for n in range(NB)]
        total = len(phases) * JT

        # phase (0,0): V units at odd slots; K/Q 512-col parts at even
        # slots, ordered by when the S/AV streams need them (K cols c*512
        # feed S slots j = 4c..4c+3).
        for jj in range(8):
            addx(2 * jj + 1,
                 lambda jj=jj, e=(jj % 2 == 1): v_unit(jj, dve_cast=e))
        addx(0, lambda: proj_part(Wk_sb, KT_sb, 0, 1))
        addx(2, lambda: proj_part(Wk_sb, KT_sb, 0, 2))
        addx(4, lambda: proj_part(Wk_sb, KT_sb, 0, 3))
        addx(6, lambda: proj_part(Wq_sb, QT_sb, 0, 1))
        # phases (0,1)/(0,2): remaining Q/K as 512-col parts (short PSUM
        # pool holds - a full 8-matmul unit stalls the 3-deep S rotation)
        parts = [(Wq_sb, QT_sb, 0, 2), (Wq_sb, QT_sb, 0, 3),
                 (Wq_sb, QT_sb, 1, 0), (Wq_sb, QT_sb, 1, 1),
                 (Wk_sb, KT_sb, 1, 0), (Wk_sb, KT_sb, 1, 1),
                 (Wq_sb, QT_sb, 1, 2), (Wq_sb, QT_sb, 1, 3),
                 (Wk_sb, KT_sb, 1, 2), (Wk_sb, KT_sb, 1, 3)]
        for idx, (w, t_, o, cb) in enumerate(parts):
            addx(16 + 1 + 4 * idx,
                 lambda w=w, t_=t_, o=o, cb=cb, e=(idx % 2 == 0):
                     proj_part(w, t_, o, cb, dve_cast=e))

        # O-projection of q block m during pair-1 phase m+1 (needs
        # finish(0,m) [pair-0 phases] and finish(1,m) [previous phase];
        # after r11 so they never wait on the r8 OT multiplies in-queue).
        for m in range(NB - 1):
            pbase = (4 + m + 1) * JT
            for t in range(4):
                addx(pbase + 12 + t,
                     lambda t=t, m=m, e=(t % 2 == 1):
                         o_unit(4 * m + t, dve_copy=e))

        # prelude: minimum to start the stream (first q/k 512-col parts
        # only need xT cols 0:512 = the first input DMA block)
        proj_part(Wq_sb, QT_sb, 0, 0, dve_cast=True)
        proj_part(Wk_sb, KT_sb, 0, 0, dve_cast=False)

        # flat stream: one S pair + one AV pair per slot, continuous (no
        # PE idle gaps -> HAM stays warm).  Slot r of phase p runs AV for
        # this phase's j = r-4, or the PREVIOUS phase's j = 12..15 for
        # r = 0..3.  The finish chain is emitted at slot r=3 BEFORE that
        # slot's exp so its PSUM reads jump ahead in the ACT/DVE queues
        # and the single AV buffer frees just in time for j=0 at r=4.
        def slot_av(g):
            p, r = g // JT, g % JT
            if r < AV_LAG:
                return (phases[p - 1], 12 + r) if p > 0 else None
            return (phases[p], r - AV_LAG)

        def finish(ia, na, g):
            rs, accS = av_finish_a1(ia, na)
            st = {}
            def a2():
                st["rb"] = av_finish_a2(ia, na, rs)
            addx(g + 2, a2)
            addx(g + TT_DELAY,
                 lambda: av_finish_b(ia, na, accS, st["rb"]))

        for g in range(total):
            i, n = phases[g // JT]
            sp = s_pair(i, n, g % JT)
            avx = slot_av(g)
            pre_exp = avx is not None and avx[1] == JT - 1
            if pre_exp:
                (ia, na), j = avx
                av(ia, na, j, 0)
                av(ia, na, j, 1)
                finish(ia, na, g)
            s_exp(i, n, g % JT, sp)
            if avx is not None and not pre_exp:
                (ia, na), j = avx
                av(ia, na, j, 0)
                av(ia, na, j, 1)
            for fn in extras.get(g, []):
                fn()

        # ---- tail ----
        # Pre-start c=0 (pair-0 OT, ready since phase 3) of three block-3
        # output projections on the freeing S pool; their accumulation
        # groups stay open across the AV drain (different PSUM banks).
        pre = []
        for t in (12, 13, 14):
            op = big.tile([P, 2, 512], F32, tag="big", name=f"op{t}")
            nc.tensor.matmul(op[:, 0, :], lhsT=OT_sb[:, 0, ts(t, P)],
                             rhs=Wo_sb[:, 0, :], start=True, stop=False,
                             skip_group_check=True)
            pre.append((t, op))
        # drain last phase's AVs
        ia, na = phases[-1]
        for j in (12, 13, 14, 15):
            av(ia, na, j, 0)
            av(ia, na, j, 1)
        # fast finish: no SBUF evacuation (nothing else needs the AV
        # buffer), recip immediately, OT multiplies straight from PSUM
        acc = av_tiles.pop((ia, na))
        rs = small.tile([1, 2, 512], F32, tag="rs", name="rs_tail")
        nc.scalar.activation(rs[:], acc[DH : DH + 1, :, :],
                             mybir.ActivationFunctionType.Copy)
        rc = small.tile([1, 2, 512], F32, tag="rc", name="rc_tail")
        nc.vector.reciprocal_approx_fast(rc[:], rs[:])
        rb = small.tile([DH, 2, 512], F32, tag="rb", name="rb_tail")
        nc.gpsimd.partition_broadcast(rb[:], rc[:])
        for s in range(2):
            nc.vector.tensor_mul(
                OT_sb[64 * s : 64 * s + DH, ia, ts(na, 512)],
                acc[0:DH, s, :],
                rb[:, s, :],
            )
        for g in range(total, total + TT_DELAY + 2):
            for fn in extras.get(g, []):
                fn()
        # complete the pre-started units (c=1 = fresh pair-1 OT) + o15
        for k, (t, op) in enumerate(pre):
            nc.tensor.matmul(op[:, 0, :], lhsT=OT_sb[:, 1, ts(t, P)],
                             rhs=Wo_sb[:, 1, :], start=False, stop=True,
                             skip_group_check=True)
            ot = outp.tile([P, D], F32, tag="out")
            if k % 2 == 1:
                nc.vector.tensor_copy(ot[:], op[:, 0, :])
            else:
                nc.scalar.activation(ot[:], op[:, 0, :],
                                     mybir.ActivationFunctionType.Copy)
            nc.sync.dma_start(out_r[:, t, :], ot[:])
        o_unit(15, dve_copy=True)

    if finalize:
        nc.finalize()
    return nc


_NC_CACHE = None


def _get_nc():
    global _NC_CACHE
    if _NC_CACHE is None:
        _NC_CACHE = build_nc()
    return _NC_CACHE


def _chunked(w):
    """[512, M] -> [128, 4, M] with row r at [r % 128, r // 128]."""
    m = w.shape[1]
    return np.ascontiguousarray(
        w.reshape(w.shape[0] // P, P, m).transpose(1, 0, 2)
    )


def make_in_maps(x, Wq, Wkv, Wo, bo):
    bf = ml_dtypes.bfloat16
    Wq = np.asarray(Wq, np.float32)
    Wkv = np.asarray(Wkv, np.float32)
    Wo = np.asarray(Wo, np.float32)
    xTs = []
    for b in range(4):
        xTs.append(_chunked(np.asarray(x[b], np.float32).T).astype(bf))
    in_maps = []
    for c in range(8):
        b, hh = divmod(c, 2)
        cols = slice(hh * 256, (hh + 1) * 256)
        in_maps.append({
            "xT": xTs[b],
            "Wq": _chunked(Wq[:, cols]).astype(bf),
            "Wk": _chunked(Wkv[:, :D][:, cols]).astype(bf),
            "Wv": _chunked(Wkv[:, D:][:, cols]).astype(bf),
            "Wo": _chunked(Wo[hh * 256 : (hh + 1) * 256, :]).astype(bf),
        })
    return in_maps


def gather_out(results, x, bo):
    b_total = x.shape[0]
    bo = np.asarray(bo, np.float32)
    out = np.empty((b_total, N, D), np.float32)
    for b in range(b_total):
        out[b] = results[2 * b]["out"] + results[2 * b + 1]["out"] + bo
    return out


def kernel(x, Wq, Wkv, Wo, bo, trace=False):
    nc = _get_nc()
    in_maps = make_in_maps(x, Wq, Wkv, Wo, bo)
    res = run_bass_kernel_spmd(nc, in_maps, core_ids=list(range(8)), trace=trace)
    out = gather_out(res.results, np.asarray(x), bo)
    if trace:
        kernel.last_exec_time_ns = res.exec_time_ns
    return out


kernel.last_exec_time_ns = None
